# revision 19
# baseline (speedup 1.0000x reference)
"""Trainium2 Bass kernel for AttentionAggregationNN (ragged single-query MHA pooling).

Algebraic reduction: with one shared query vector, softmax-attention pooling per
group collapses to, per instance i and head h:
    e[i,h]   = exp(s_h . x_i)          (softmax shift-invariance drops the
                                        constant logit term)
    val[i,h] = t_h . x_i
    out[g]   = sum_h (sum_{i in g} e*val) / (sum_{i in g} e) + CONST
where s_h = Wk_h^T q_h / sqrt(D), t_h = Wv_h^T (w_lin @ w_out)_h, folded on the
host in float64.

Device work per core (data-parallel over groups, host pre-sorts by group):
  mm1: SP[128,16] = XT_subtile.T @ W16            (scores | vals), f32 PSUM
       X is fp8e4m3 (stationary operand -> fast FWL weight loads, half DMA)
  ACT: e = exp(scores);  DVE: ev = e * vals       (batched over 16 subtiles)
  one-hot M[i,j] = (rel_gid[i] == j)              (one DVE op per batch,
                                                   j-iota generated on device)
  mm2: acc[16, win] += [e|ev].T @ M               (segment sums into PSUM)
  epilogue per group-half: transpose, recip*mul, reduce, transpose to [1,128],
  +CONST; both halves land in one [1,256] SBUF row -> single-descriptor DMA.

Each core owns 256 contiguous groups, split into two halves of 128 groups; each
half's rows are padded to a multiple of 128. x0/x1 (the two E-halves) stream via
separate dma_starts on the sync/scalar rings so descriptor generation (~2.5ns
per descriptor, 128 descriptors per dma_start) runs in parallel.
"""
import os

if os.environ.get("AXON_H4_ENABLED") == "1" or os.environ.get("AXON_TERMINAL_JOB_NAME"):
    plats = os.environ.get("JAX_PLATFORMS", "")
    if "axon" not in plats:
        os.environ["JAX_PLATFORMS"] = "axon,cpu"

import numpy as np

# ---------------------------------------------------------------- problem dims
N, G, E, H, D = 131072, 2048, 256, 8, 32
NCORES = 8
GC = G // NCORES        # 256 groups per core
HC = GC // 2            # 128 groups per half
P = 128                 # partition dim / subtile rows
BATCH = 16              # subtiles per ACT/DVE batch
HALF_QUANT = 128        # row padding quantum per half (= P)

USE_FP8 = True
PIPELINED_MM2 = True        # emit mm2 one batch behind mm1 (hide ACT->DVE chain)

_CACHE: dict = {}


# ---------------------------------------------------------------- host algebra
def _fold_params(query, w_in, b_in, w_out, b_out, w_lin, b_lin):
    q64 = query.reshape(E).astype(np.float64)
    w64, b64 = w_in.astype(np.float64), b_in.astype(np.float64)
    wq, wk, wv = w64[:E], w64[E:2 * E], w64[2 * E:]
    bq, bk, bv = b64[:E], b64[E:2 * E], b64[2 * E:]
    q = wq @ q64 + bq
    qh = q.reshape(H, D)
    S = np.einsum("hde,hd->he", wk.reshape(H, D, E), qh) / np.sqrt(D)
    u = (w_lin.astype(np.float64) @ w_out.astype(np.float64)).reshape(E)
    uh = u.reshape(H, D)
    T = np.einsum("hde,hd->he", wv.reshape(H, D, E), uh)
    const = float(np.einsum("hd,hd->", uh, bv.reshape(H, D))
                  + w_lin.astype(np.float64).reshape(E) @ b_out.astype(np.float64)
                  + b_lin.astype(np.float64)[0])
    W16 = np.concatenate([S.T, T.T], axis=1)    # [E, 16]
    return W16, const


def _shard_prep(tree_preds, group_ids, x_dtype, np_dtype):
    sizes = np.bincount(group_ids, minlength=G)
    offsets = np.concatenate([[0], np.cumsum(sizes)]).astype(np.int64)
    sorter = np.argsort(group_ids, kind="stable")
    Xs = np.ascontiguousarray(tree_preds[sorter])
    gs = group_ids[sorter].astype(np.int64)

    # per (core, half): row range and size
    hstart = offsets[(np.arange(2 * NCORES) * HC)]
    hend = offsets[(np.arange(2 * NCORES) + 1) * HC]
    hrows = (hend - hstart).reshape(NCORES, 2)
    caps = [int(np.ceil(hrows[:, h].max() / HALF_QUANT) * HALF_QUANT) for h in (0, 1)]
    rows_cap = caps[0] + caps[1]
    nsub = rows_cap // P
    nsub_half = [caps[0] // P, caps[1] // P]

    # relative gid within half, -1 for pad rows
    grel = np.full((NCORES, rows_cap), -1, np.int64)
    for c in range(NCORES):
        for h in (0, 1):
            i = 2 * c + h
            n = hend[i] - hstart[i]
            col0 = h * caps[0]
            grel[c, col0:col0 + n] = gs[hstart[i]:hend[i]] - (c * GC + h * HC)
    gsub = grel.reshape(NCORES, nsub, P)
    lo = np.where(gsub >= 0, gsub, G).min(axis=(0, 2))
    hi = np.where(gsub >= 0, gsub, -1).max(axis=(0, 2))
    span = int(np.where(hi >= 0, hi - np.minimum(lo, hi) + 1, 1).max())
    win = 32 if span <= 28 else (64 if span <= 60 else 128)
    assert span <= win, f"one-hot span {span} > {win}"
    woff = np.minimum(np.where(lo < G, lo, 0), HC - win).astype(np.int64)
    assert ((hi < woff + win) | (hi < 0)).all()

    rel = np.where(gsub >= 0, gsub - woff[None, :, None], win).astype(np.float32)
    assert ((rel >= 0) & (rel <= win)).all()
    RELT = np.ascontiguousarray(rel.transpose(0, 2, 1)).astype(np_dtype)  # [NC,P,nsub]

    XT = np.zeros((NCORES, 2, P, rows_cap), x_dtype)
    for c in range(NCORES):
        for h in (0, 1):
            i = 2 * c + h
            n = hend[i] - hstart[i]
            col0 = h * caps[0]
            blk = Xs[hstart[i]:hend[i]].T.astype(x_dtype)
            XT[c, 0, :, col0:col0 + n] = blk[:P]
            XT[c, 1, :, col0:col0 + n] = blk[P:]
    return XT, RELT, woff, caps, nsub_half, win


def _macro_schedule(cap, first_half, last_half):
    """DMA macro-tile sizes covering `cap` rows. Descriptor generation costs
    ~600ns per dma_start on the issuing sequencer regardless of size, so use
    few, large macros; the first transfers happen during the NEFF preamble
    (~5.5us) when the PE can't run anyway. Last macro smallish so the tail
    (PE work after the final DMA lands) is short."""
    head = [2048]
    tail = [1024] if last_half else []
    mid = cap - sum(head) - sum(tail)
    assert mid >= 0, f"cap {cap} too small for schedule"
    sizes = (head + [4096] * (mid // 4096)
             + ([mid % 4096] if mid % 4096 else []) + tail)
    assert sum(sizes) == cap and all(s % HALF_QUANT == 0 for s in sizes)
    return sizes


# ---------------------------------------------------------------- bass program
def _build_program(caps, nsub_half, woff, const, win):
    import concourse.bass as bass
    import concourse.tile as tile
    from concourse import bacc, mybir

    DT = mybir.dt.bfloat16
    XDT = mybir.dt.float8e4 if USE_FP8 else mybir.dt.bfloat16
    F32 = mybir.dt.float32
    Exp = mybir.ActivationFunctionType.Exp
    Alu = mybir.AluOpType
    rows_cap = caps[0] + caps[1]
    nsub = nsub_half[0] + nsub_half[1]
    JW = BATCH * win

    nc = bacc.Bacc(None, target_bir_lowering=False)
    xt = nc.dram_tensor("xt", [2, P, rows_cap], XDT, kind="ExternalInput")
    # consts: [ w0(16) | w1(16) | relt(nsub) ]  (bf16)
    consts = nc.dram_tensor("consts", [P, 32 + nsub], DT, kind="ExternalInput")
    out = nc.dram_tensor("out", [1, GC], F32, kind="ExternalOutput")

    with tile.TileContext(nc) as tc:
        with (
            tc.tile_pool(name="const", bufs=1) as constp,
            tc.tile_pool(name="x0p", bufs=6) as x0p,
            tc.tile_pool(name="x1p", bufs=6) as x1p,
            tc.tile_pool(name="work", bufs=8) as workp,
            tc.tile_pool(name="ep", bufs=1) as epsb,
            tc.tile_pool(name="mm1", bufs=5, space="PSUM") as mm1p,
            tc.tile_pool(name="acc", bufs=1, space="PSUM") as accp,
            tc.tile_pool(name="tps", bufs=1, space="PSUM") as tpsp,
        ):
            # ---- constants. One small DMA on the scalar ring; everything else
            # is generated on idle engines (gpsimd iota/memset, vector memset)
            # so the sync ring starts the big x loads immediately.
            cb_t = constp.tile([P, 32 + nsub], DT)
            nc.scalar.dma_start(cb_t[:], consts[:])
            w0 = cb_t[:, 0:16]
            w1 = cb_t[:, 16:32]
            RT = 32                      # start of relt columns inside cb_t

            # j-iota [P, JW]: value j % win, identical on every partition
            jt = constp.tile([P, JW], DT)
            nc.gpsimd.iota(jt[:], [[0, BATCH], [1, win]], channel_multiplier=0,
                           allow_small_or_imprecise_dtypes=True)
            # zero [P,128] bf16 stationary for PSUM-clearing matmuls
            zw = constp.tile([P, P], DT)
            nc.vector.memset(zw[:], 0.0)
            # ones [8,1] f32 stationary for the heads-reduce... not needed;
            # heads are reduced on the free axis. ident128 f32 for the final
            # [128,1] -> [1,128] transpose-matmul, built as (row == col).
            pi_t = constp.tile([P, 1], F32)
            nc.gpsimd.iota(pi_t[:], [[0, 1]], channel_multiplier=1,
                           allow_small_or_imprecise_dtypes=True)
            jr_t = constp.tile([P, P], F32)
            nc.gpsimd.iota(jr_t[:], [[1, P]], channel_multiplier=0,
                           allow_small_or_imprecise_dtypes=True)
            ident = constp.tile([P, P], F32)
            nc.vector.tensor_tensor(ident[:], jr_t[:],
                                    pi_t[:].to_broadcast([P, P]),
                                    op=Alu.is_equal)

            # Three accumulator strips per half at partition bases 0/32/64
            # (quadrant 3 is unusable per HW erratum): subtile s accumulates
            # into strip s%3. The inferred tile_position=(0, 32k) packs the
            # mm2 matmuls into disjoint column groups of the PE array so they
            # run concurrently.
            accB = accp.tile([P, 2 * HC], F32, tag="accB", name="accB")
            accs = [accB[:, 0:HC], accB[:, HC:2 * HC]]
            nc.tensor.matmul(accB[:, 0:2 * HC], lhsT=zw, rhs=jt[:, 0:2 * HC],
                             start=True, stop=False, skip_group_check=True)

            # final outputs accumulate into one [1, 256] SBUF row; single
            # contiguous DMA descriptor at the very end.
            oo2 = epsb.tile([1, GC], F32, tag="oo2")

            def epilogue(h):
                acc = accs[h]
                cc = epsb.tile([P, HC], F32, tag=f"cc{h}")
                nc.vector.tensor_copy(cc[:], acc[:])
                tpb = tpsp.tile([P, P], F32, tag="tpb")
                nc.tensor.transpose(tpb[:], cc[:], ident[:])
                # sum the 4 strips: cols [32k, 32k+8) are e-sums, [32k+8,
                # 32k+16) are ev-sums of strip k
                cs = epsb.tile([P, 80], F32, tag=f"cs{h}")
                nc.vector.tensor_copy(cs[:], tpb[:, 0:80])
                e01 = epsb.tile([P, 16], F32, tag=f"e01{h}")
                nc.vector.tensor_tensor(e01[:], cs[:, 0:16], cs[:, 32:48],
                                        op=Alu.add)
                es = epsb.tile([P, 16], F32, tag=f"es{h}")
                nc.vector.tensor_tensor(es[:], e01[:], cs[:, 64:80], op=Alu.add)
                rec = epsb.tile([P, 8], F32, tag=f"rec{h}")
                nc.vector.reciprocal(rec[:], es[:, 0:8])
                rr = epsb.tile([P, 8], F32, tag=f"rr{h}")
                nc.vector.tensor_tensor(rr[:], rec[:], es[:, 8:16], op=Alu.mult)
                oo1 = epsb.tile([P, 1], F32, tag=f"oo1{h}")
                nc.vector.tensor_reduce(oo1[:], rr[:], axis=mybir.AxisListType.X,
                                        op=Alu.add)
                # [128,1] -> [1,128] via matmul with the identity (f32), then
                # +const lands the half into its slot of oo2.
                opst = tpsp.tile([1, P], F32, tag="opst")
                nc.tensor.matmul(opst[:], lhsT=oo1[:], rhs=ident[:],
                                 start=True, stop=True, skip_group_check=True)
                nc.vector.tensor_scalar_add(oo2[0:1, h * HC:(h + 1) * HC],
                                            opst[:], float(const))

            # ---- main loop. mm2s are emitted one batch late so the PE never
            # stalls on the ACT->DVE chain of the batch it just produced.
            pending = None          # (sp2, m_t, s0, bsz, h)

            # last subtile index using quad strip k, per half (for stop flags)
            ranges = [(0, nsub_half[0]), (nsub_half[0], nsub_half[0] + nsub_half[1])]
            last_q = [{k: max(s for s in range(b, e) if s % 3 == k)
                       for k in range(3)} for (b, e) in ranges]

            def flush_pending():
                nonlocal pending
                if pending is None:
                    return
                sp2, m_t, s0, bsz, ph = pending
                acc = accs[ph]
                for j in range(bsz):
                    s_i = s0 + j
                    k = s_i % 3
                    nc.tensor.matmul(
                        acc[32 * k:32 * k + 16, woff[s_i]:woff[s_i] + win],
                        lhsT=sp2[:, j * 16:j * 16 + 16],
                        rhs=m_t[:, j * win:(j + 1) * win],
                        start=False, stop=(s_i == last_q[ph][k]),
                        skip_group_check=True)
                pending = None

            # flat macro list across both halves; x0 streams issue on the sync
            # ring (dedicated), x1 on the scalar ring with a +3 macro lookahead
            # hoisted ahead of the EXPs in program order so descriptor
            # generation (~600ns per dma_start) never stalls the stream.
            macros = []
            for h in (0, 1):
                m0 = 0 if h == 0 else caps[0]
                for msz in _macro_schedule(caps[h], first_half=(h == 0),
                                           last_half=(h == 1)):
                    macros.append((h, m0, msz))
                    m0 += msz
            x0_t, x1_t = {}, {}

            def issue(i):
                if i >= len(macros):
                    return
                _, im0, imsz = macros[i]
                x0_t[i] = x0p.tile([P, 4096], XDT, tag="x0", name=f"x0_{i}")
                nc.sync.dma_start(x0_t[i][:, 0:imsz], xt[0, :, im0:im0 + imsz])
                x1_t[i] = x1p.tile([P, 4096], XDT, tag="x1", name=f"x1_{i}")
                nc.gpsimd.dma_start(x1_t[i][:, 0:imsz], xt[1, :, im0:im0 + imsz])

            for i in range(3):
                issue(i)
            s = 0
            cur_h = 0
            for mi, (h, m0, msz) in enumerate(macros):
                if h != cur_h:
                    flush_pending()
                    epilogue(cur_h)
                    cur_h = h
                issue(mi + 3)
                x0, x1 = x0_t.pop(mi), x1_t.pop(mi)
                if True:
                    b0 = 0
                    while b0 < msz:
                        bsz = min(BATCH, (msz - b0) // P)      # subtiles in batch
                        spp = mm1p.tile([P, 16 * BATCH], F32)
                        m_t = workp.tile([P, BATCH * win], DT, tag="m")
                        sp2 = workp.tile([P, 16 * BATCH], DT, tag="sp2")
                        for j in range(bsz):
                            col = b0 + j * P
                            nc.tensor.matmul(spp[:, j * 16:j * 16 + 16],
                                             lhsT=x0[:, col:col + P],
                                             rhs=w0,
                                             start=True, stop=False)
                            nc.tensor.matmul(spp[:, j * 16:j * 16 + 16],
                                             lhsT=x1[:, col:col + P],
                                             rhs=w1,
                                             start=False, stop=True)
                        if PIPELINED_MM2:
                            flush_pending()
                        # batched one-hot: M[i, b, w] = (rel[i, s+b] == w)
                        mv = m_t[:].rearrange("p (b w) -> p b w", w=win)
                        jv = jt[:, 0:bsz * win].rearrange("p (b w) -> p b w", w=win)
                        relb = cb_t[:, RT + s:RT + s + bsz].to_broadcast([P, bsz, win])
                        nc.vector.tensor_tensor(mv[:, 0:bsz, :], jv, relb,
                                                op=Alu.is_equal)
                        spv = spp[:].rearrange("p (b c) -> p b c", c=16)
                        sp2v = sp2[:].rearrange("p (b c) -> p b c", c=16)
                        nc.scalar.activation(sp2v[:, 0:bsz, 0:8], spv[:, 0:bsz, 0:8],
                                             Exp)
                        nc.vector.tensor_tensor(sp2v[:, 0:bsz, 8:16],
                                                sp2v[:, 0:bsz, 0:8],
                                                spv[:, 0:bsz, 8:16], op=Alu.mult)
                        pending = (sp2, m_t, s, bsz, h)
                        if not PIPELINED_MM2:
                            flush_pending()
                        s += bsz
                        b0 += bsz * P
            flush_pending()
            epilogue(1)
            nc.sync.dma_start(out[0:1, 0:GC], oo2[:])
    nc.compile()
    return nc


# ---------------------------------------------------------------- entry point
def _invoke(tree_preds, group_ids, query, w_in, b_in, w_out, b_out, w_lin, b_lin,
            trace=False, **spmd_kwargs):
    import ml_dtypes
    np_dt = ml_dtypes.bfloat16
    x_dt = ml_dtypes.float8_e4m3 if USE_FP8 else ml_dtypes.bfloat16

    tree_preds = np.asarray(tree_preds, dtype=np.float32)
    group_ids = np.asarray(group_ids, dtype=np.int32)

    W16, const = _fold_params(np.asarray(query), np.asarray(w_in), np.asarray(b_in),
                              np.asarray(w_out), np.asarray(b_out),
                              np.asarray(w_lin), np.asarray(b_lin))
    XT, RELT, woff, caps, nsub_half, win = _shard_prep(tree_preds, group_ids,
                                                       x_dt, np_dt)

    key = (tuple(caps), tuple(nsub_half), tuple(woff.tolist()), float(const), win,
           USE_FP8, PIPELINED_MM2)
    if _CACHE.get("key") != key:
        _CACHE["nc"] = _build_program(caps, nsub_half, woff, const, win)
        _CACHE["key"] = key
    nc = _CACHE["nc"]

    nsub = sum(nsub_half)
    # consts: [ w0(16) | w1(16) | relt(nsub) ]
    cb = np.zeros((NCORES, P, 32 + nsub), np_dt)
    wmat = W16.astype(np_dt).reshape(2, P, 16)
    cb[:, :, 0:16] = wmat[0]
    cb[:, :, 16:32] = wmat[1]
    cb[:, :, 32:] = RELT

    in_maps = [{"xt": XT[c], "consts": np.ascontiguousarray(cb[c])}
               for c in range(NCORES)]

    from concourse.bass_utils import run_bass_kernel_spmd
    res = run_bass_kernel_spmd(nc, in_maps, core_ids=list(range(NCORES)),
                               trace=trace, **spmd_kwargs)

    out = np.empty((G, 1), np.float32)
    for c in range(NCORES):
        out[c * GC:(c + 1) * GC, 0] = res.results[c]["out"][0]
    return out, res


def kernel(tree_preds, group_ids, query, w_in, b_in, w_out, b_out, w_lin, b_lin):
    out, _ = _invoke(tree_preds, group_ids, query, w_in, b_in,
                     w_out, b_out, w_lin, b_lin)
    return out
